# revision 29
# baseline (speedup 1.0000x reference)
"""TRN2 Bass kernel for nn_CMAT_4561255269047 (dual-stream CNN + cross-attention).

Data-parallel over batch B=8 across 8 NeuronCores (1 sample/core, no collectives).

Per-core program (all matmuls fp32r at full PE rate):
  conv3x3 = 9 shifted matmuls over zero-padded [C,46,46] images, accumulated in
  PSUM over input-channel chunks (ci-outer loop, 8 PSUM banks resident).
  conv1 -> BN+ReLU fused into the PSUM-drain activation (scale/bias APs).
  conv2 -> gated residual relu((o2w+b)*o1 + (o2b+b)) via scalar_tensor_tensor.
  attention (all-fp8e4m3): q/k quantized with a folded -ln16 shift in spare
  row 32 (so exp fits e4m3 range; the shift cancels in feat/Z), sT, feat and
  Z all run as fp8 DoubleRow matmuls; eT = exp(sT) cast to fp8 by ACT. The
  Z-ones lhsT is M=128 so Z lands on all partitions and 1/Z needs no
  broadcast. Normalize, residual add, DMA out per chunk. Both attentions
  share one software-pipelined emission stream. gate*beta / (1-gate)*gamma
  are folded into vw/vb on the host.
"""
import sys
sys.path.insert(0, '/opt/trn_rl_repo')

import numpy as np
import ml_dtypes

import concourse.bass as bass
import concourse.mybir as mybir
import concourse.tile as tile
from concourse import bacc
from concourse.bass_utils import run_bass_kernel_spmd

MM_KINDS = {}

F32 = mybir.dt.float32
F32R = mybir.dt.float32r
BF16 = mybir.dt.bfloat16
F8E4 = mybir.dt.float8e4
F8E5 = mybir.dt.float8e5
PM_DR = mybir.MatmulPerfMode.DoubleRow
BF16_CONV = True  # bf16 convs save ~27us; 6e-3 rel err passes the 2e-2 gate
CONV_DT = BF16 if BF16_CONV else F32R
EPS = 1e-5
AF = mybir.ActivationFunctionType
ALU = mybir.AluOpType

H = W = 44
HP = WP = 46
N = H * W            # 1936
NCH = 4              # spatial n-chunks of 11 rows (484 px) for convs / att m
ROWS = 11
PX = ROWS * W        # 484
AJ = 16              # attention n-chunks of 128 (last = 16)
NK = 2048            # fp8 q/k padded length (16*128; cols N:NK zero)
ESHIFT = 2.7725887   # ln(16): shifts scores so exp fits fp8e4m3 (max 448)

# prm packed-param columns
C_BNS1, C_BNT1, C_BNS2, C_BNT2 = 0, 2, 4, 6
C_C2B1, C_C2B2 = 8, 12
C_QB1, C_KB1, C_QB2, C_KB2 = 16, 17, 18, 19
C_VB1, C_VB2 = 20, 22            # v-bias as per-partition scalars, 2 c-chunks each
C_ONESR, C_ONESC = 24, 152       # ones row (partition 0) / ones column
C_ZERO = 153                     # 46 zero cols (o1p border source)
C_ZEROW = 200                    # 484 zero cols (K-padding source)
PRM_COLS = 684


def _mm(nc, kind, *args, **kw):
    inst = nc.tensor.matmul(*args, **kw)
    try:
        MM_KINDS[inst.ins.name] = kind
    except Exception:
        pass
    return inst


def _bg_step(bg, n):
    """Pop two background thunks once >=8 conv matmuls have passed since the
    last pop: keeps the exp drain rate (~1 per 0.8us) while paying the PE's
    fp8<->bf16 mode-switch cost once per pair."""
    if bg is None:
        return
    bg['since'] += n
    if bg['q'] and bg['since'] >= 8:
        bg['since'] = 0
        bg['q'].popleft()()
        if bg['q']:
            bg['q'].popleft()()


def _conv_stream(nc, tc, x_d, w1_d, w2_d, bns_col, bnt_col, c2b_col,
                 prm_t, o1p_t, out_t, wpool, xpool, cps, ctmp, zero_borders,
                 bg=None, on_round=None):
    """One sa_block: conv1 -> BN+relu -> o1p_t (padded), conv2 + gating -> out_t.

    Both convs hold only 4 PSUM accumulators at a time (conv1 is mch-outer,
    conv2 runs one 11-row n-chunk per round) so 2 banks stay free for the
    background sT/exp stream threaded through `bg`.
    """
    f32 = lambda ap: ap.bitcast(F32)

    if zero_borders:
        # zero the o1p padding ring once (interior is fully overwritten per stream)
        zsrc = prm_t[:, C_ZERO:C_ZERO + HP]
        for ci in range(2):
            nc.vector.tensor_copy(o1p_t[:, ci, 0, :], zsrc)
            nc.vector.tensor_copy(o1p_t[:, ci, HP - 1, :], zsrc)
            nc.vector.tensor_copy(o1p_t[:, ci, :, 0], zsrc)
            nc.vector.tensor_copy(o1p_t[:, ci, :, HP - 1], zsrc)

    # ---- conv1: Cin=512 (4 ci chunks) -> C=256 (2 m chunks, sequential) ----
    xpcs = []
    for mch in range(2):
        psums = {}
        for nch in range(NCH):
            psums[nch] = cps.tile([128, PX], F32, tag="cps", name=f"c1p_{mch}_{nch}")
        for ci in range(4):
            if mch == 0:
                # interleave x DMAs with the first mch round's weight DMAs so
                # the first matmul only waits on (xpc0, w1c0)
                xpc = xpool.tile([128, HP, WP], CONV_DT, tag="xpad")
                xsrc = x_d[ci] if BF16_CONV else x_d[ci].bitcast(F32R)
                nc.sync.dma_start(xpc[:, 0:23, :], xsrc[:, 0:23, :])
                nc.sync.dma_start(xpc[:, 23:HP, :], xsrc[:, 23:HP, :])
                xpcs.append(xpc)
            w1c = wpool.tile([128, 9, 128], CONV_DT, tag="w1")
            wsrc = (w1_d[ci] if BF16_CONV else w1_d[ci].bitcast(F32R))[:, :, 128 * mch:128 * (mch + 1)]
            nc.sync.dma_start(w1c[:, 0:5, :], wsrc[:, 0:5, :])
            nc.sync.dma_start(w1c[:, 5:9, :], wsrc[:, 5:9, :])
            for dy in range(3):
                for dx in range(3):
                    # n-chunk innermost: 4 consecutive matmuls reuse the same lhsT
                    for nch in range(NCH):
                        _mm(nc, "conv1",
                            psums[nch][:],
                            w1c[:, 3 * dy + dx, :],
                            xpcs[ci][:, ROWS * nch + dy:ROWS * nch + dy + ROWS, dx:dx + W],
                            start=(ci == 0 and dy == 0 and dx == 0),
                            stop=(ci == 3 and dy == 2 and dx == 2),
                            skip_group_check=True,
                        )
                    _bg_step(bg, 4)
        for nch in range(NCH):
            # o1 = relu(conv * bn_scale + bn_shift), written into padded interior
            nc.scalar.activation(
                o1p_t[:, mch, 1 + ROWS * nch:1 + ROWS * (nch + 1), 1:1 + W],
                psums[nch][:].rearrange("p (a b) -> p a b", a=ROWS),
                AF.Relu,
                bias=f32(prm_t[:, bnt_col + mch:bnt_col + mch + 1]),
                scale=f32(prm_t[:, bns_col + mch:bns_col + mch + 1]),
            )

    # ---- conv2: C=256 (2 ci chunks) -> 2C=512 (4 m chunks), 1 n-chunk/round ----
    for rd in range(NCH):
        p2 = {}
        for m in range(4):
            p2[m] = cps.tile([128, PX], F32, tag="cps", name=f"c2p_{rd}_{m}")
        for ci in range(2):
            w2c = wpool.tile([128, 9, 512], CONV_DT, tag="w")
            nc.sync.dma_start(w2c[:], w2_d[ci] if BF16_CONV else w2_d[ci].bitcast(F32R))
            for m in range(4):
                for dy in range(3):
                    for dx in range(3):
                        _mm(nc, "conv2",
                            p2[m][:],
                            w2c[:, 3 * dy + dx, 128 * m:128 * (m + 1)],
                            o1p_t[:, ci, ROWS * rd + dy:ROWS * rd + dy + ROWS, dx:dx + W],
                            start=(ci == 0 and dy == 0 and dx == 0),
                            stop=(ci == 1 and dy == 2 and dx == 2),
                            skip_group_check=True,
                        )
                        _bg_step(bg, 1)
        o1_int = o1p_t if BF16_CONV else f32(o1p_t)
        for mch in range(2):
            pw = p2[mch][:].rearrange("p (a b) -> p a b", a=ROWS)
            pb = p2[mch + 2][:].rearrange("p (a b) -> p a b", a=ROWS)
            t1 = ctmp.tile([128, ROWS, W], F32, tag="g1")
            # t1 = (o2w + c2b_w) * o1
            nc.vector.scalar_tensor_tensor(
                t1[:], pw, f32(prm_t[:, c2b_col + mch:c2b_col + mch + 1]),
                o1_int[:, mch, 1 + ROWS * rd:1 + ROWS * (rd + 1), 1:1 + W],
                ALU.add, ALU.mult)
            t2 = ctmp.tile([128, ROWS, W], F32, tag="g2")
            # t2 = (o2b + c2b_b) + t1
            nc.vector.scalar_tensor_tensor(
                t2[:], pb, f32(prm_t[:, c2b_col + mch + 2:c2b_col + mch + 3]),
                t1[:], ALU.add, ALU.add)
            nc.scalar.activation(
                out_t[:, mch, PX * rd:PX * (rd + 1)].rearrange("p (a b) -> p a b", a=ROWS),
                t2[:], AF.Relu)
        if on_round is not None:
            on_round(rd)


def _att_weights(nc, qkw_d, vw_d, pool, tags):
    qkw_t = pool.tile([128, 2, 64], F32R, tag=tags + "qkw", name=tags + "qkw")
    vw_t = pool.tile([128, 2, 256], F32R, tag=tags + "vw", name=tags + "vw")
    for kc in range(2):
        nc.sync.dma_start(qkw_t[:, kc, :], qkw_d[kc].bitcast(F32R))
        nc.sync.dma_start(vw_t[:, kc, :], vw_d[kc].bitcast(F32R))
    return qkw_t, vw_t


def _att_qk_alloc(nc, prm_t, pool, tags):
    """Allocate q/k fp8 [128, 2, NK] (two K-planes for DoubleRow sT; plane 1
    and all rows 32:128 stay zero). Row 32 of plane 0 carries (q=-ESHIFT, k=1)
    so sT picks up a constant -ESHIFT and exp fits e4m3 range.
    Emitted early: the fills have no data deps beyond prm."""
    zw = prm_t[:, C_ZEROW:C_ZEROW + PX]
    zw32 = zw.bitcast(mybir.dt.uint32)
    q_t = pool.tile([128, 2, NK], F8E4, tag=tags + "q", name=tags + "q")
    k_t = pool.tile([128, 2, NK], F8E4, tag=tags + "k", name=tags + "k")
    for t in (q_t, k_t):
        t32 = t[:].bitcast(mybir.dt.uint32)  # [128, 2, NK/4] zero fill
        for p in range(2):
            nc.vector.tensor_copy(t32[:, p, 0:484], zw32[:, :])
            nc.vector.tensor_copy(t32[:, p, 484:512], zw32[:, 0:28])
    for im in range(NCH):
        msl = slice(PX * im, PX * (im + 1))
        nc.vector.tensor_scalar_add(q_t[32:33, 0, msl], zw[0:1, :].bitcast(F32), -ESHIFT)
        nc.vector.tensor_scalar_add(k_t[32:33, 0, msl], zw[0:1, :].bitcast(F32), 1.0)
    return q_t, k_t


def _att_qk_im(nc, qkw_t, qb_col, kb_col, src_qk, prm_t, q_t, k_t, pspool, tags, im):
    """Fill q,k plane-0 rows 0:32 for one 484-col chunk (fp8e4 cast)."""
    f32 = lambda ap: ap.bitcast(F32)
    msl = slice(PX * im, PX * (im + 1))
    pq = pspool.tile([64, PX], F32, tag="prep", name=tags + f"pq{im}")
    for kc in range(2):
        _mm(nc, 'qk', pq[:], qkw_t[:, kc, :], src_qk[:, kc, msl],
            start=(kc == 0), stop=(kc == 1), skip_group_check=True)
    nc.vector.tensor_scalar_add(q_t[0:32, 0, msl], pq[0:32, :], f32(prm_t[0:32, qb_col:qb_col + 1]))
    nc.vector.tensor_scalar_add(k_t[0:32, 0, msl], pq[32:64, :], f32(prm_t[0:32, kb_col:kb_col + 1]))


def _att_qk(nc, qkw_t, qb_col, kb_col, src_qk, prm_t, q_t, k_t, pspool, tags):
    for im in range(NCH):
        _att_qk_im(nc, qkw_t, qb_col, kb_col, src_qk, prm_t, q_t, k_t, pspool, tags, im)


def _att_v_alloc(nc, prm_t, pool, tags):
    """vT [n, c] fp8e4; zero the (partially-filled) last chunk upfront."""
    zw = prm_t[:, C_ZEROW:C_ZEROW + PX].bitcast(F32)
    vT_t = pool.tile([128, AJ, 256], F8E4, tag=tags + "vT", name=tags + "vT")
    nc.vector.tensor_copy(vT_t[:, AJ - 1, :], zw[:, 0:256])
    return vT_t


def _att_v_chunk(nc, vw_t, src_v, vT_t, pspool, tags, jn):
    """One vT chunk: pv[n, c] = src_v^T @ vw, cast to fp8e4."""
    nsz = 128 if jn < AJ - 1 else 16
    pv = pspool.tile([128, 256], F32, tag="prep", name=tags + f"pv{jn}")
    for kc in range(2):
        _mm(nc, 'vT', pv[0:nsz, :],
            src_v[:, kc, 128 * jn:128 * jn + nsz],
            vw_t[:, kc, :],
            start=(kc == 0), stop=(kc == 1), skip_group_check=True)
    nc.vector.tensor_copy(vT_t[0:nsz, jn, :], pv[0:nsz, :])


def _att_make_emitters(nc, atts, prm_t, pstp, epool, ones8, holder, dbg=None):
    """Emitters shared by the background (prefetch) stream and the final phase.

    emit_st(ai, im, jn): fp8 DoubleRow sT matmul (pstp psum) + ACT exp -> eT.
    emit_feat(ai, im, j): paired DoubleRow feat/z matmuls (holder['fz'] psum);
    at j==7 emits normalize + residual + DMA out (holder['atmp'] sbuf).
    """
    f32 = lambda ap: ap.bitcast(F32)
    zw = prm_t[:, C_ZEROW:C_ZEROW + PX].bitcast(F32)
    eT, pf, pz = {}, {}, {}
    holder['eT'] = eT

    def emit_out(ai, im):
        atmp = holder['atmp']
        vb_col, res_t, out_d = atts[ai][3], atts[ai][4], atts[ai][5]
        msl = slice(PX * im, PX * (im + 1))
        izb = atmp.tile([128, PX], F32, tag="izb", bufs=2, name=f"izb{ai}_{im}")
        nc.vector.reciprocal(izb[:], pz[(ai, im)][:])
        for cch in range(2):
            fo = atmp.tile([128, PX], F32, tag="fo", bufs=4, name=f"fo{ai}_{im}_{cch}")
            nc.vector.tensor_mul(fo[:], pf[(ai, im)][cch][:], izb[:])
            oo = atmp.tile([128, PX], F32, tag="oo", bufs=4, name=f"oo{ai}_{im}_{cch}")
            # out = (feat/Z + vb) + r   (v-bias folded here: sum(mask)=1)
            nc.vector.scalar_tensor_tensor(
                oo[:], fo[:], f32(prm_t[:, vb_col + cch:vb_col + cch + 1]),
                f32(res_t[:, cch, msl]), ALU.add, ALU.add)
            nc.sync.dma_start(out_d[cch, :, msl], oo[:])
        if dbg is not None and (ai, im) == (0, 0):
            nc.sync.dma_start(dbg['dbgz'][:], izb[:])
            nc.sync.dma_start(dbg['dbge'][:], eT[(0, 0)][:])

    def emit_st(ai, im, jn):
        q_t, k_t = atts[ai][0], atts[ai][1]
        if jn == 0:
            # a2 (ai=1) eT chunks are prefetched during the depth convs and
            # must all stay live; a1 rotates through 2 buffers.
            eT[(ai, im)] = epool.tile([128, AJ, PX], F8E4, tag=f"eT{ai}",
                                      bufs=(4 if ai == 1 else 2), name=f"eT{ai}_{im}")
            nc.vector.tensor_copy(eT[(ai, im)][:, AJ - 1, :], zw[:, :])
        msl = slice(PX * im, PX * (im + 1))
        nsz = 128 if jn < AJ - 1 else 16
        pst = pstp.tile([128, PX], F32, tag="st", name=f"pst_{ai}_{im}_{jn}")
        _mm(nc, 'sT', pst[:],
            k_t[:, :, 128 * jn:128 * (jn + 1)],
            q_t[:, :, msl],
            start=True, stop=True, perf_mode=PM_DR, skip_group_check=True)
        nc.scalar.activation(eT[(ai, im)][0:nsz, jn, :], pst[0:nsz, :], AF.Exp)

    def emit_feat(ai, im, j):
        fz = holder['fz']
        vT_t = atts[ai][2]
        e = eT[(ai, im)]
        if j == 0:
            pf[(ai, im)] = (fz.tile([128, PX], F32, tag="f0", name=f"pf0_{ai}_{im}"),
                            fz.tile([128, PX], F32, tag="f1", name=f"pf1_{ai}_{im}"))
            pz[(ai, im)] = fz.tile([128, PX], F32, tag="z", name=f"pz_{ai}_{im}")
        st, sp = (j == 0), (j == 7)
        for cch in range(2):
            _mm(nc, 'feat', pf[(ai, im)][cch][:],
                vT_t[:, 2 * j:2 * j + 2, 128 * cch:128 * (cch + 1)],
                e[:, 2 * j:2 * j + 2, :],
                start=st, stop=sp, perf_mode=PM_DR, skip_group_check=True)
        _mm(nc, 'z', pz[(ai, im)][:], ones8[:], e[:, 2 * j:2 * j + 2, :],
            start=st, stop=sp, perf_mode=PM_DR, skip_group_check=True)
        if sp:
            emit_out(ai, im)

    return emit_st, emit_feat


DEBUG_DUMP = False


def build_nc():
    nc = bacc.Bacc(None)
    d = {}
    cdt = CONV_DT if BF16_CONV else F32
    d['xr'] = nc.dram_tensor("xr", [4, 128, HP, WP], cdt, kind="ExternalInput")
    d['xd'] = nc.dram_tensor("xd", [4, 128, HP, WP], cdt, kind="ExternalInput")
    d['w1r'] = nc.dram_tensor("w1r", [4, 128, 9, 256], cdt, kind="ExternalInput")
    d['w2r'] = nc.dram_tensor("w2r", [2, 128, 9, 512], cdt, kind="ExternalInput")
    d['w1d'] = nc.dram_tensor("w1d", [4, 128, 9, 256], cdt, kind="ExternalInput")
    d['w2d'] = nc.dram_tensor("w2d", [2, 128, 9, 512], cdt, kind="ExternalInput")
    for a in (1, 2):
        d[f'qkw{a}'] = nc.dram_tensor(f"qkw{a}", [2, 128, 64], F32, kind="ExternalInput")
        d[f'vw{a}'] = nc.dram_tensor(f"vw{a}", [2, 128, 256], F32, kind="ExternalInput")
    d['prm'] = nc.dram_tensor("prm", [128, PRM_COLS], F32, kind="ExternalInput")
    d['o1'] = nc.dram_tensor("o1", [2, 128, N], F32, kind="ExternalOutput")
    d['o2'] = nc.dram_tensor("o2", [2, 128, N], F32, kind="ExternalOutput")
    if DEBUG_DUMP:
        d['dbgq'] = nc.dram_tensor("dbgq", [128, N], F32, kind="ExternalOutput")
        d['dbgk'] = nc.dram_tensor("dbgk", [128, N], F32, kind="ExternalOutput")
        d['dbgv'] = nc.dram_tensor("dbgv", [128, AJ, 256], F8E4, kind="ExternalOutput")
        d['dbgz'] = nc.dram_tensor("dbgz", [128, PX], F32, kind="ExternalOutput")
        d['dbge'] = nc.dram_tensor("dbge", [128, AJ, PX], F8E4, kind="ExternalOutput")
        d['dbgr'] = nc.dram_tensor("dbgr", [128, 2, N], F32, kind="ExternalOutput")
        d['dbgd'] = nc.dram_tensor("dbgd", [128, 2, N], F32, kind="ExternalOutput")

    with tile.TileContext(nc) as tc:
        with tc.tile_pool(name="persist", bufs=1) as persist, \
             tc.tile_pool(name="aearly", bufs=1) as aearly:
            prm_t = persist.tile([128, PRM_COLS], F32R, tag="prm")
            nc.sync.dma_start(prm_t[:], d['prm'][:].bitcast(F32R))
            r_t = persist.tile([128, 2, N], F32R, tag="r")
            d_t = persist.tile([128, 2, N], F32R, tag="d")
            ones8 = persist.tile([128, 2, 128], F8E4, tag="ones8")
            for j in range(2):
                nc.vector.tensor_scalar_add(
                    ones8[:, j, :], prm_t[:, C_ZEROW:C_ZEROW + 128].bitcast(F32), 1.0)

            qkw1_t, vw1_t = _att_weights(nc, d['qkw1'], d['vw1'], aearly, "a1")
            qkw2_t, vw2_t = _att_weights(nc, d['qkw2'], d['vw2'], aearly, "a2")
            q1_t, k1_t = _att_qk_alloc(nc, prm_t, aearly, "a1")
            q2_t, k2_t = _att_qk_alloc(nc, prm_t, aearly, "a2")
            vT1_t = _att_v_alloc(nc, prm_t, aearly, "a1")
            vT2_t = _att_v_alloc(nc, prm_t, aearly, "a2")
            atts = [(q1_t, k1_t, vT1_t, C_VB1, r_t, d['o1']),
                    (q2_t, k2_t, vT2_t, C_VB2, d_t, d['o2'])]
            holder = {}

            with tc.tile_pool(name="pstp", bufs=2, space="PSUM") as pstp, \
                 tc.tile_pool(name="epool", bufs=2) as epool, \
                 tc.tile_pool(name="atmp", bufs=2) as atmp:
                holder['atmp'] = atmp
                emit_st, emit_feat = _att_make_emitters(
                    nc, atts, prm_t, pstp, epool, ones8, holder,
                    dbg=d if DEBUG_DUMP else None)

                with tc.tile_pool(name="wpool", bufs=3) as wpool, \
                     tc.tile_pool(name="xpool", bufs=4) as xpool, \
                     tc.tile_pool(name="o1pool", bufs=1) as o1pool, \
                     tc.tile_pool(name="cps", bufs=4, space="PSUM") as cps, \
                     tc.tile_pool(name="prep", bufs=2, space="PSUM") as prep, \
                     tc.tile_pool(name="ctmp", bufs=3) as ctmp:
                    o1p_t = o1pool.tile([128, 2, HP, WP], CONV_DT, tag="o1p")
                    _conv_stream(nc, tc, d['xr'], d['w1r'], d['w2r'],
                                 C_BNS1, C_BNT1, C_C2B1, prm_t, o1p_t, r_t,
                                 wpool, xpool, cps, ctmp, True)
                    # rgb-dependent preps: a1 v from r, a2 q/k from r
                    for jn in range(AJ):
                        _att_v_chunk(nc, vw1_t, r_t, vT1_t, prep, "a1", jn)
                    _att_qk(nc, qkw2_t, C_QB2, C_KB2, r_t, prm_t, q2_t, k2_t, prep, "a2")

                    # a2's sT/exp stream + vT2 chunks thread through the depth
                    # convs (2 free PSUM banks; exps use idle ACT time there)
                    from collections import deque
                    bg = {'q': deque(), 'since': 0}
                    for im in range(NCH):
                        for jn in range(AJ):
                            bg['q'].append(lambda im=im, jn=jn: emit_st(1, im, jn))

                    done_st = set()

                    def st_thunk(ai, im, jn):
                        done_st.add((ai, im, jn))
                        return lambda: emit_st(ai, im, jn)

                    def on_round(rd):
                        lo = 0 if rd == 0 else (PX * rd) // 128
                        hi = (PX * (rd + 1)) // 128 if rd < NCH - 1 else AJ
                        for jn in range(lo, hi):
                            bg['q'].append(
                                lambda jn=jn: _att_v_chunk(nc, vw2_t, d_t, vT2_t,
                                                           prep, "a2", jn))
                        # d cols for round rd are ready: project q1/k1 chunk rd
                        # and schedule the a1 sTs whose q/k chunks now exist
                        bg['q'].append(
                            lambda rd=rd: _att_qk_im(nc, qkw1_t, C_QB1, C_KB1, d_t,
                                                     prm_t, q1_t, k1_t, prep, "a1", rd))
                        # only ims 0..1: eT0 has 2 buffers, and im>=2 would
                        # cycle-wait on im0's final-phase feat reads
                        if rd < NCH - 1:
                            for im in range(min(rd + 1, 2)):
                                for jn in range(hi):
                                    if (0, im, jn) not in done_st:
                                        bg['q'].append(st_thunk(0, im, jn))

                    _conv_stream(nc, tc, d['xd'], d['w1d'], d['w2d'],
                                 C_BNS2, C_BNT2, C_C2B2, prm_t, o1p_t, d_t,
                                 wpool, xpool, cps, ctmp, False,
                                 bg=bg, on_round=on_round)
                    while bg['q']:
                        bg['q'].popleft()()

                # conv/prep PSUM freed; feat/z accumulators take their place
                with tc.tile_pool(name="fzp", bufs=2, space="PSUM") as fzp:
                    holder['fz'] = fzp
                    if DEBUG_DUMP:
                        nc.sync.dma_start(d['dbgq'][:], q1_t[:].bitcast(F32))
                        nc.sync.dma_start(d['dbgk'][:], k1_t[:].bitcast(F32))
                        nc.sync.dma_start(d['dbgv'][:], vT1_t[:])
                        nc.sync.dma_start(d['dbgr'][:], r_t[:].bitcast(F32))
                        nc.sync.dma_start(d['dbgd'][:], d_t[:].bitcast(F32))
                    # final phase: remaining sT/exp stream (ims not fully
                    # prefetched) paced 1:1 with feat blocks. a2 blocks and any
                    # a1 pair whose two sTs already ran are ready at slot 0;
                    # the rest join as their rem sTs emit.
                    from collections import deque
                    rem = [(0, im, jn) for im in range(NCH) for jn in range(AJ)
                           if (0, im, jn) not in done_st]
                    rem_idx = {t: i for i, t in enumerate(rem)}
                    ready_now, pending = [], []
                    for im in range(NCH):
                        for j in range(8):
                            need = [(0, im, 2 * j), (0, im, 2 * j + 1)]
                            idxs = [rem_idx[t] for t in need if t in rem_idx]
                            if idxs:
                                pending.append((max(idxs), (0, im, j)))
                            else:
                                ready_now.append((0, im, j))
                    pending.sort()
                    blocks = deque([(1, p // 8, p % 8) for p in range(32)] + ready_now)
                    for i in range(max(64, len(rem))):
                        if i < len(rem):
                            emit_st(*rem[i])
                        while pending and pending[0][0] <= i - 2:
                            blocks.append(pending.pop(0)[1])
                        if blocks:
                            emit_feat(*blocks.popleft())
                    while pending:
                        blocks.append(pending.pop(0)[1])
                    while blocks:
                        emit_feat(*blocks.popleft())

    nc.finalize()
    return nc


def _prep_common(g):
    """Host-side weight layout prep (shared across cores)."""
    out = {}
    for pre, kw1, kw2 in (('sa1', 'w1r', 'w2r'), ('sa2', 'w1d', 'w2d')):
        c1w = g[f'{pre}_c1_w']  # [256, 512, 3, 3]
        c2w = g[f'{pre}_c2_w']  # [512, 256, 3, 3]
        cnp = ml_dtypes.bfloat16 if BF16_CONV else np.float32
        out[kw1] = np.ascontiguousarray(
            c1w.transpose(1, 2, 3, 0).reshape(4, 128, 9, 256).astype(cnp))
        out[kw2] = np.ascontiguousarray(
            c2w.transpose(1, 2, 3, 0).reshape(2, 128, 9, 512).astype(cnp))

    gate = float(g['gate'][0]); beta = float(g['beta'][0]); gamma = float(g['gamma'][0])
    s1 = gate * beta
    s2 = (1.0 - gate) * gamma
    for a, s in ((1, s1), (2, s2)):
        vw = (s * g[f'a{a}_vw']).astype(np.float32)
        qkw = np.concatenate([g[f'a{a}_qw'], g[f'a{a}_kw']], axis=0)  # [64, 256]
        out[f'qkw{a}'] = np.ascontiguousarray(qkw.T.reshape(2, 128, 64))
        out[f'vw{a}'] = np.ascontiguousarray(vw.T.reshape(2, 128, 256))

    prm = np.zeros((128, PRM_COLS), np.float32)
    for pre, cs, ct, cb in (('sa1', C_BNS1, C_BNT1, C_C2B1), ('sa2', C_BNS2, C_BNT2, C_C2B2)):
        s = (g[f'{pre}_bn_g'] / np.sqrt(g[f'{pre}_bn_v'] + EPS)).astype(np.float32)
        t = ((g[f'{pre}_c1_b'] - g[f'{pre}_bn_m']) * s + g[f'{pre}_bn_b']).astype(np.float32)
        prm[:, cs:cs + 2] = s.reshape(2, 128).T
        prm[:, ct:ct + 2] = t.reshape(2, 128).T
        prm[:, cb:cb + 4] = g[f'{pre}_c2_b'].reshape(4, 128).T
    prm[0:32, C_QB1] = g['a1_qb']; prm[0:32, C_KB1] = g['a1_kb']
    prm[0:32, C_QB2] = g['a2_qb']; prm[0:32, C_KB2] = g['a2_kb']
    prm[:, C_VB1:C_VB1 + 2] = (s1 * g['a1_vb']).astype(np.float32).reshape(2, 128).T
    prm[:, C_VB2:C_VB2 + 2] = (s2 * g['a2_vb']).astype(np.float32).reshape(2, 128).T
    prm[0, C_ONESR:C_ONESR + 128] = 1.0
    prm[:, C_ONESC] = 1.0
    out['prm'] = prm
    return out


def _prep_x(x):
    """[512, 44, 44] -> padded [4, 128, 46, 46]."""
    p = np.zeros((512, HP, WP), ml_dtypes.bfloat16 if BF16_CONV else np.float32)
    p[:, 1:45, 1:45] = x
    return p.reshape(4, 128, HP, WP)


_NC_CACHE = None


def kernel(**inputs):
    global _NC_CACHE
    g = {k: np.asarray(v, np.float32) for k, v in inputs.items()}
    if _NC_CACHE is None:
        _NC_CACHE = build_nc()
    nc = _NC_CACHE

    common = _prep_common(g)
    B = g['rgb'].shape[0]
    in_maps = []
    for b in range(B):
        m = dict(common)
        m['xr'] = _prep_x(g['rgb'][b])
        m['xd'] = _prep_x(g['depth'][b])
        in_maps.append(m)

    res = run_bass_kernel_spmd(nc, in_maps, list(range(B)))
    out1 = np.stack([res.results[b]['o1'].reshape(256, H, W) for b in range(B)])
    out2 = np.stack([res.results[b]['o2'].reshape(256, H, W) for b in range(B)])
    return out1, out2



# revision 30
# speedup vs baseline: 1.0077x; 1.0077x over previous
"""TRN2 Bass kernel for nn_CMAT_4561255269047 (dual-stream CNN + cross-attention).

Data-parallel over batch B=8 across 8 NeuronCores (1 sample/core, no collectives).

Per-core program:
  conv3x3 (bf16, 1 col/cycle PE rate) = 9 shifted matmuls over zero-padded
  [C,46,46] images, PSUM-accumulated over input-channel chunks. conv1 runs
  mch-outer and conv2 one 11-row n-chunk per round so only 4 PSUM banks are
  held, leaving 2 banks for a background attention stream threaded through
  the conv tap loops: all of a2's sT/exp chain (its q/k depend only on the
  first stream) plus vT2/qk1/early-a1 chunks execute during the depth convs,
  hiding their ACT exp time under PE-bound conv phases.
  conv1 -> BN+ReLU fused into the PSUM-drain activation (scale/bias APs).
  conv2 -> gated residual relu((o2w+b)*o1 + (o2b+b)) via scalar_tensor_tensor.
  attention (all-fp8e4m3): q/k quantized with a folded -ln16 shift in spare
  row 32 (so exp fits e4m3 range; the shift cancels in feat/Z), sT, feat and
  Z all run as fp8 DoubleRow matmuls; eT = exp(sT) cast to fp8 by ACT. The
  Z-ones lhsT is M=128 so Z lands on all partitions and 1/Z needs no
  broadcast. Normalize, residual add, DMA out per chunk. Both attentions
  share one software-pipelined emission stream. gate*beta / (1-gate)*gamma
  are folded into vw/vb on the host.
"""
import sys
sys.path.insert(0, '/opt/trn_rl_repo')

import numpy as np
import ml_dtypes

import concourse.bass as bass
import concourse.mybir as mybir
import concourse.tile as tile
from concourse import bacc
from concourse.bass_utils import run_bass_kernel_spmd

MM_KINDS = {}

F32 = mybir.dt.float32
F32R = mybir.dt.float32r
BF16 = mybir.dt.bfloat16
F8E4 = mybir.dt.float8e4
F8E5 = mybir.dt.float8e5
PM_DR = mybir.MatmulPerfMode.DoubleRow
BF16_CONV = True  # bf16 convs save ~27us; 6e-3 rel err passes the 2e-2 gate
CONV_DT = BF16 if BF16_CONV else F32R
EPS = 1e-5
AF = mybir.ActivationFunctionType
ALU = mybir.AluOpType

H = W = 44
HP = WP = 46
N = H * W            # 1936
NCH = 4              # spatial n-chunks of 11 rows (484 px) for convs / att m
ROWS = 11
PX = ROWS * W        # 484
AJ = 16              # attention n-chunks of 128 (last = 16)
NK = 2048            # fp8 q/k padded length (16*128; cols N:NK zero)
ESHIFT = 2.7725887   # ln(16): shifts scores so exp fits fp8e4m3 (max 448)

# prm packed-param columns
C_BNS1, C_BNT1, C_BNS2, C_BNT2 = 0, 2, 4, 6
C_C2B1, C_C2B2 = 8, 12
C_QB1, C_KB1, C_QB2, C_KB2 = 16, 17, 18, 19
C_VB1, C_VB2 = 20, 22            # v-bias as per-partition scalars, 2 c-chunks each
C_ONESR, C_ONESC = 24, 152       # ones row (partition 0) / ones column
C_ZERO = 153                     # 46 zero cols (o1p border source)
C_ZEROW = 200                    # 484 zero cols (K-padding source)
PRM_COLS = 684


def _mm(nc, kind, *args, **kw):
    inst = nc.tensor.matmul(*args, **kw)
    try:
        MM_KINDS[inst.ins.name] = kind
    except Exception:
        pass
    return inst


def _bg_step(bg, n):
    """Pop two background thunks once >=8 conv matmuls have passed since the
    last pop: keeps the exp drain rate (~1 per 0.8us) while paying the PE's
    fp8<->bf16 mode-switch cost once per pair."""
    if bg is None:
        return
    bg['since'] += n
    if bg['q'] and bg['since'] >= 8:
        bg['since'] = 0
        bg['q'].popleft()()
        if bg['q']:
            bg['q'].popleft()()


def _conv_stream(nc, tc, x_d, w1_d, w2_d, bns_col, bnt_col, c2b_col,
                 prm_t, o1p_t, out_t, wpool, xpool, cps, ctmp, zero_borders,
                 bg=None, on_round=None):
    """One sa_block: conv1 -> BN+relu -> o1p_t (padded), conv2 + gating -> out_t.

    Both convs hold only 4 PSUM accumulators at a time (conv1 is mch-outer,
    conv2 runs one 11-row n-chunk per round) so 2 banks stay free for the
    background sT/exp stream threaded through `bg`.
    """
    f32 = lambda ap: ap.bitcast(F32)

    if zero_borders:
        # zero the o1p padding ring once (interior is fully overwritten per stream)
        zsrc = prm_t[:, C_ZERO:C_ZERO + HP]
        for ci in range(2):
            nc.vector.tensor_copy(o1p_t[:, ci, 0, :], zsrc)
            nc.vector.tensor_copy(o1p_t[:, ci, HP - 1, :], zsrc)
            nc.vector.tensor_copy(o1p_t[:, ci, :, 0], zsrc)
            nc.vector.tensor_copy(o1p_t[:, ci, :, HP - 1], zsrc)

    # ---- conv1: Cin=512 (4 ci chunks) -> C=256 (2 m chunks, sequential) ----
    xpcs = []
    for mch in range(2):
        psums = {}
        for nch in range(NCH):
            psums[nch] = cps.tile([128, PX], F32, tag="cps", name=f"c1p_{mch}_{nch}")
        for ci in range(4):
            if mch == 0:
                # interleave x DMAs with the first mch round's weight DMAs so
                # the first matmul only waits on (xpc0, w1c0)
                xpc = xpool.tile([128, HP, WP], CONV_DT, tag="xpad")
                xsrc = x_d[ci] if BF16_CONV else x_d[ci].bitcast(F32R)
                nc.sync.dma_start(xpc[:, 0:23, :], xsrc[:, 0:23, :])
                nc.sync.dma_start(xpc[:, 23:HP, :], xsrc[:, 23:HP, :])
                xpcs.append(xpc)
            w1c = wpool.tile([128, 9, 128], CONV_DT, tag="w1")
            wsrc = (w1_d[ci] if BF16_CONV else w1_d[ci].bitcast(F32R))[:, :, 128 * mch:128 * (mch + 1)]
            nc.sync.dma_start(w1c[:, 0:5, :], wsrc[:, 0:5, :])
            nc.sync.dma_start(w1c[:, 5:9, :], wsrc[:, 5:9, :])
            for dy in range(3):
                for dx in range(3):
                    # n-chunk innermost: 4 consecutive matmuls reuse the same lhsT
                    for nch in range(NCH):
                        _mm(nc, "conv1",
                            psums[nch][:],
                            w1c[:, 3 * dy + dx, :],
                            xpcs[ci][:, ROWS * nch + dy:ROWS * nch + dy + ROWS, dx:dx + W],
                            start=(ci == 0 and dy == 0 and dx == 0),
                            stop=(ci == 3 and dy == 2 and dx == 2),
                            skip_group_check=True,
                        )
                    _bg_step(bg, 4)
        for nch in range(NCH):
            # o1 = relu(conv * bn_scale + bn_shift), written into padded interior
            nc.scalar.activation(
                o1p_t[:, mch, 1 + ROWS * nch:1 + ROWS * (nch + 1), 1:1 + W],
                psums[nch][:].rearrange("p (a b) -> p a b", a=ROWS),
                AF.Relu,
                bias=f32(prm_t[:, bnt_col + mch:bnt_col + mch + 1]),
                scale=f32(prm_t[:, bns_col + mch:bns_col + mch + 1]),
            )

    # ---- conv2: C=256 (2 ci chunks) -> 2C=512 (4 m chunks), 1 n-chunk/round ----
    for rd in range(NCH):
        p2 = {}
        for m in range(4):
            p2[m] = cps.tile([128, PX], F32, tag="cps", name=f"c2p_{rd}_{m}")
        for ci in range(2):
            w2c = wpool.tile([128, 9, 512], CONV_DT, tag="w")
            nc.sync.dma_start(w2c[:], w2_d[ci] if BF16_CONV else w2_d[ci].bitcast(F32R))
            for m in range(4):
                for dy in range(3):
                    for dx in range(3):
                        _mm(nc, "conv2",
                            p2[m][:],
                            w2c[:, 3 * dy + dx, 128 * m:128 * (m + 1)],
                            o1p_t[:, ci, ROWS * rd + dy:ROWS * rd + dy + ROWS, dx:dx + W],
                            start=(ci == 0 and dy == 0 and dx == 0),
                            stop=(ci == 1 and dy == 2 and dx == 2),
                            skip_group_check=True,
                        )
                        _bg_step(bg, 1)
        o1_int = o1p_t if BF16_CONV else f32(o1p_t)
        for mch in range(2):
            pw = p2[mch][:].rearrange("p (a b) -> p a b", a=ROWS)
            pb = p2[mch + 2][:].rearrange("p (a b) -> p a b", a=ROWS)
            t1 = ctmp.tile([128, ROWS, W], F32, tag="g1")
            # t1 = (o2w + c2b_w) * o1
            nc.vector.scalar_tensor_tensor(
                t1[:], pw, f32(prm_t[:, c2b_col + mch:c2b_col + mch + 1]),
                o1_int[:, mch, 1 + ROWS * rd:1 + ROWS * (rd + 1), 1:1 + W],
                ALU.add, ALU.mult)
            t2 = ctmp.tile([128, ROWS, W], F32, tag="g2")
            # t2 = (o2b + c2b_b) + t1
            nc.vector.scalar_tensor_tensor(
                t2[:], pb, f32(prm_t[:, c2b_col + mch + 2:c2b_col + mch + 3]),
                t1[:], ALU.add, ALU.add)
            nc.scalar.activation(
                out_t[:, mch, PX * rd:PX * (rd + 1)].rearrange("p (a b) -> p a b", a=ROWS),
                t2[:], AF.Relu)
        if on_round is not None:
            on_round(rd)


def _att_weights(nc, qkw_d, vw_d, pool, tags):
    qkw_t = pool.tile([128, 2, 64], F32R, tag=tags + "qkw", name=tags + "qkw")
    vw_t = pool.tile([128, 2, 256], F32R, tag=tags + "vw", name=tags + "vw")
    for kc in range(2):
        nc.sync.dma_start(qkw_t[:, kc, :], qkw_d[kc].bitcast(F32R))
        nc.sync.dma_start(vw_t[:, kc, :], vw_d[kc].bitcast(F32R))
    return qkw_t, vw_t


def _att_qk_alloc(nc, prm_t, pool, tags):
    """Allocate q/k fp8 [128, 2, NK] (two K-planes for DoubleRow sT; plane 1
    and all rows 32:128 stay zero). Row 32 of plane 0 carries (q=-ESHIFT, k=1)
    so sT picks up a constant -ESHIFT and exp fits e4m3 range.
    Emitted early: the fills have no data deps beyond prm."""
    zw = prm_t[:, C_ZEROW:C_ZEROW + PX]
    zw32 = zw.bitcast(mybir.dt.uint32)
    q_t = pool.tile([128, 2, NK], F8E4, tag=tags + "q", name=tags + "q")
    k_t = pool.tile([128, 2, NK], F8E4, tag=tags + "k", name=tags + "k")
    for t in (q_t, k_t):
        t32 = t[:].bitcast(mybir.dt.uint32)  # [128, 2, NK/4] zero fill
        for p in range(2):
            nc.vector.tensor_copy(t32[:, p, 0:484], zw32[:, :])
            nc.vector.tensor_copy(t32[:, p, 484:512], zw32[:, 0:28])
    for im in range(NCH):
        msl = slice(PX * im, PX * (im + 1))
        nc.vector.tensor_scalar_add(q_t[32:33, 0, msl], zw[0:1, :].bitcast(F32), -ESHIFT)
        nc.vector.tensor_scalar_add(k_t[32:33, 0, msl], zw[0:1, :].bitcast(F32), 1.0)
    return q_t, k_t


def _att_qk_im(nc, qkw_t, qb_col, kb_col, src_qk, prm_t, q_t, k_t, pspool, tags, im):
    """Fill q,k plane-0 rows 0:32 for one 484-col chunk (fp8e4 cast)."""
    f32 = lambda ap: ap.bitcast(F32)
    msl = slice(PX * im, PX * (im + 1))
    pq = pspool.tile([64, PX], F32, tag="prep", name=tags + f"pq{im}")
    for kc in range(2):
        _mm(nc, 'qk', pq[:], qkw_t[:, kc, :], src_qk[:, kc, msl],
            start=(kc == 0), stop=(kc == 1), skip_group_check=True)
    nc.vector.tensor_scalar_add(q_t[0:32, 0, msl], pq[0:32, :], f32(prm_t[0:32, qb_col:qb_col + 1]))
    nc.vector.tensor_scalar_add(k_t[0:32, 0, msl], pq[32:64, :], f32(prm_t[0:32, kb_col:kb_col + 1]))


def _att_qk(nc, qkw_t, qb_col, kb_col, src_qk, prm_t, q_t, k_t, pspool, tags):
    for im in range(NCH):
        _att_qk_im(nc, qkw_t, qb_col, kb_col, src_qk, prm_t, q_t, k_t, pspool, tags, im)


def _att_v_alloc(nc, prm_t, pool, tags):
    """vT [n, c] fp8e4; zero the (partially-filled) last chunk upfront."""
    zw = prm_t[:, C_ZEROW:C_ZEROW + PX].bitcast(F32)
    vT_t = pool.tile([128, AJ, 256], F8E4, tag=tags + "vT", name=tags + "vT")
    nc.vector.tensor_copy(vT_t[:, AJ - 1, :], zw[:, 0:256])
    return vT_t


def _att_v_chunk(nc, vw_t, src_v, vT_t, pspool, tags, jn):
    """One vT chunk: pv[n, c] = src_v^T @ vw, cast to fp8e4."""
    nsz = 128 if jn < AJ - 1 else 16
    pv = pspool.tile([128, 256], F32, tag="prep", name=tags + f"pv{jn}")
    for kc in range(2):
        _mm(nc, 'vT', pv[0:nsz, :],
            src_v[:, kc, 128 * jn:128 * jn + nsz],
            vw_t[:, kc, :],
            start=(kc == 0), stop=(kc == 1), skip_group_check=True)
    nc.vector.tensor_copy(vT_t[0:nsz, jn, :], pv[0:nsz, :])


def _att_make_emitters(nc, atts, prm_t, pstp, epool, ones8, holder, dbg=None):
    """Emitters shared by the background (prefetch) stream and the final phase.

    emit_st(ai, im, jn): fp8 DoubleRow sT matmul (pstp psum) + ACT exp -> eT.
    emit_feat(ai, im, j): paired DoubleRow feat/z matmuls (holder['fz'] psum);
    at j==7 emits normalize + residual + DMA out (holder['atmp'] sbuf).
    """
    f32 = lambda ap: ap.bitcast(F32)
    zw = prm_t[:, C_ZEROW:C_ZEROW + PX].bitcast(F32)
    eT, pf, pz = {}, {}, {}
    holder['eT'] = eT

    def emit_out(ai, im):
        atmp = holder['atmp']
        vb_col, res_t, out_d = atts[ai][3], atts[ai][4], atts[ai][5]
        msl = slice(PX * im, PX * (im + 1))
        izb = atmp.tile([128, PX], F32, tag="izb", bufs=2, name=f"izb{ai}_{im}")
        nc.vector.reciprocal(izb[:], pz[(ai, im)][:])
        for cch in range(2):
            fo = atmp.tile([128, PX], F32, tag="fo", bufs=4, name=f"fo{ai}_{im}_{cch}")
            nc.vector.tensor_mul(fo[:], pf[(ai, im)][cch][:], izb[:])
            oo = atmp.tile([128, PX], F32, tag="oo", bufs=4, name=f"oo{ai}_{im}_{cch}")
            # out = (feat/Z + vb) + r   (v-bias folded here: sum(mask)=1)
            nc.vector.scalar_tensor_tensor(
                oo[:], fo[:], f32(prm_t[:, vb_col + cch:vb_col + cch + 1]),
                f32(res_t[:, cch, msl]), ALU.add, ALU.add)
            nc.sync.dma_start(out_d[cch, :, msl], oo[:])
        if dbg is not None and (ai, im) == (0, 0):
            nc.sync.dma_start(dbg['dbgz'][:], izb[:])
            nc.sync.dma_start(dbg['dbge'][:], eT[(0, 0)][:])

    def emit_st(ai, im, jn):
        q_t, k_t = atts[ai][0], atts[ai][1]
        if jn == 0:
            # a2 (ai=1) eT chunks are prefetched during the depth convs and
            # must all stay live; a1 rotates through 2 buffers.
            eT[(ai, im)] = epool.tile([128, AJ, PX], F8E4, tag=f"eT{ai}",
                                      bufs=(4 if ai == 1 else 2), name=f"eT{ai}_{im}")
            nc.vector.tensor_copy(eT[(ai, im)][:, AJ - 1, :], zw[:, :])
        msl = slice(PX * im, PX * (im + 1))
        nsz = 128 if jn < AJ - 1 else 16
        pst = pstp.tile([128, PX], F32, tag="st", name=f"pst_{ai}_{im}_{jn}")
        _mm(nc, 'sT', pst[:],
            k_t[:, :, 128 * jn:128 * (jn + 1)],
            q_t[:, :, msl],
            start=True, stop=True, perf_mode=PM_DR, skip_group_check=True)
        nc.scalar.activation(eT[(ai, im)][0:nsz, jn, :], pst[0:nsz, :], AF.Exp)

    def emit_feat(ai, im, j):
        fz = holder['fz']
        vT_t = atts[ai][2]
        e = eT[(ai, im)]
        if j == 0:
            pf[(ai, im)] = (fz.tile([128, PX], F32, tag="f0", name=f"pf0_{ai}_{im}"),
                            fz.tile([128, PX], F32, tag="f1", name=f"pf1_{ai}_{im}"))
            pz[(ai, im)] = fz.tile([128, PX], F32, tag="z", name=f"pz_{ai}_{im}")
        st, sp = (j == 0), (j == 7)
        for cch in range(2):
            _mm(nc, 'feat', pf[(ai, im)][cch][:],
                vT_t[:, 2 * j:2 * j + 2, 128 * cch:128 * (cch + 1)],
                e[:, 2 * j:2 * j + 2, :],
                start=st, stop=sp, perf_mode=PM_DR, skip_group_check=True)
        _mm(nc, 'z', pz[(ai, im)][:], ones8[:], e[:, 2 * j:2 * j + 2, :],
            start=st, stop=sp, perf_mode=PM_DR, skip_group_check=True)
        if sp:
            emit_out(ai, im)

    return emit_st, emit_feat


DEBUG_DUMP = False


def build_nc():
    nc = bacc.Bacc(None)
    d = {}
    cdt = CONV_DT if BF16_CONV else F32
    d['xr'] = nc.dram_tensor("xr", [4, 128, HP, WP], cdt, kind="ExternalInput")
    d['xd'] = nc.dram_tensor("xd", [4, 128, HP, WP], cdt, kind="ExternalInput")
    d['w1r'] = nc.dram_tensor("w1r", [4, 128, 9, 256], cdt, kind="ExternalInput")
    d['w2r'] = nc.dram_tensor("w2r", [2, 128, 9, 512], cdt, kind="ExternalInput")
    d['w1d'] = nc.dram_tensor("w1d", [4, 128, 9, 256], cdt, kind="ExternalInput")
    d['w2d'] = nc.dram_tensor("w2d", [2, 128, 9, 512], cdt, kind="ExternalInput")
    for a in (1, 2):
        d[f'qkw{a}'] = nc.dram_tensor(f"qkw{a}", [2, 128, 64], F32, kind="ExternalInput")
        d[f'vw{a}'] = nc.dram_tensor(f"vw{a}", [2, 128, 256], F32, kind="ExternalInput")
    d['prm'] = nc.dram_tensor("prm", [128, PRM_COLS], F32, kind="ExternalInput")
    d['o1'] = nc.dram_tensor("o1", [2, 128, N], F32, kind="ExternalOutput")
    d['o2'] = nc.dram_tensor("o2", [2, 128, N], F32, kind="ExternalOutput")
    if DEBUG_DUMP:
        d['dbgq'] = nc.dram_tensor("dbgq", [128, N], F32, kind="ExternalOutput")
        d['dbgk'] = nc.dram_tensor("dbgk", [128, N], F32, kind="ExternalOutput")
        d['dbgv'] = nc.dram_tensor("dbgv", [128, AJ, 256], F8E4, kind="ExternalOutput")
        d['dbgz'] = nc.dram_tensor("dbgz", [128, PX], F32, kind="ExternalOutput")
        d['dbge'] = nc.dram_tensor("dbge", [128, AJ, PX], F8E4, kind="ExternalOutput")
        d['dbgr'] = nc.dram_tensor("dbgr", [128, 2, N], F32, kind="ExternalOutput")
        d['dbgd'] = nc.dram_tensor("dbgd", [128, 2, N], F32, kind="ExternalOutput")

    with tile.TileContext(nc) as tc:
        with tc.tile_pool(name="persist", bufs=1) as persist, \
             tc.tile_pool(name="aearly", bufs=1) as aearly:
            prm_t = persist.tile([128, PRM_COLS], F32R, tag="prm")
            nc.sync.dma_start(prm_t[:], d['prm'][:].bitcast(F32R))
            r_t = persist.tile([128, 2, N], F32R, tag="r")
            d_t = persist.tile([128, 2, N], F32R, tag="d")
            ones8 = persist.tile([128, 2, 128], F8E4, tag="ones8")
            for j in range(2):
                nc.vector.tensor_scalar_add(
                    ones8[:, j, :], prm_t[:, C_ZEROW:C_ZEROW + 128].bitcast(F32), 1.0)

            qkw1_t, vw1_t = _att_weights(nc, d['qkw1'], d['vw1'], aearly, "a1")
            qkw2_t, vw2_t = _att_weights(nc, d['qkw2'], d['vw2'], aearly, "a2")
            q1_t, k1_t = _att_qk_alloc(nc, prm_t, aearly, "a1")
            q2_t, k2_t = _att_qk_alloc(nc, prm_t, aearly, "a2")
            vT1_t = _att_v_alloc(nc, prm_t, aearly, "a1")
            vT2_t = _att_v_alloc(nc, prm_t, aearly, "a2")
            atts = [(q1_t, k1_t, vT1_t, C_VB1, r_t, d['o1']),
                    (q2_t, k2_t, vT2_t, C_VB2, d_t, d['o2'])]
            holder = {}

            with tc.tile_pool(name="pstp", bufs=2, space="PSUM") as pstp, \
                 tc.tile_pool(name="epool", bufs=2) as epool, \
                 tc.tile_pool(name="atmp", bufs=2) as atmp:
                holder['atmp'] = atmp
                emit_st, emit_feat = _att_make_emitters(
                    nc, atts, prm_t, pstp, epool, ones8, holder,
                    dbg=d if DEBUG_DUMP else None)

                with tc.tile_pool(name="wpool", bufs=3) as wpool, \
                     tc.tile_pool(name="xpool", bufs=4) as xpool, \
                     tc.tile_pool(name="o1pool", bufs=1) as o1pool, \
                     tc.tile_pool(name="cps", bufs=4, space="PSUM") as cps, \
                     tc.tile_pool(name="prep", bufs=2, space="PSUM") as prep, \
                     tc.tile_pool(name="ctmp", bufs=3) as ctmp:
                    o1p_t = o1pool.tile([128, 2, HP, WP], CONV_DT, tag="o1p")
                    _conv_stream(nc, tc, d['xr'], d['w1r'], d['w2r'],
                                 C_BNS1, C_BNT1, C_C2B1, prm_t, o1p_t, r_t,
                                 wpool, xpool, cps, ctmp, True)
                    # rgb-dependent preps: a1 v from r, a2 q/k from r
                    for jn in range(AJ):
                        _att_v_chunk(nc, vw1_t, r_t, vT1_t, prep, "a1", jn)
                    _att_qk(nc, qkw2_t, C_QB2, C_KB2, r_t, prm_t, q2_t, k2_t, prep, "a2")

                    # a2's sT/exp stream + vT2 chunks thread through the depth
                    # convs (2 free PSUM banks; exps use idle ACT time there)
                    from collections import deque
                    bg = {'q': deque(), 'since': 0}
                    for im in range(NCH):
                        for jn in range(AJ):
                            bg['q'].append(lambda im=im, jn=jn: emit_st(1, im, jn))

                    done_st = set()

                    def st_thunk(ai, im, jn):
                        done_st.add((ai, im, jn))
                        return lambda: emit_st(ai, im, jn)

                    def on_round(rd):
                        lo = 0 if rd == 0 else (PX * rd) // 128
                        hi = (PX * (rd + 1)) // 128 if rd < NCH - 1 else AJ
                        for jn in range(lo, hi):
                            bg['q'].append(
                                lambda jn=jn: _att_v_chunk(nc, vw2_t, d_t, vT2_t,
                                                           prep, "a2", jn))
                        # d cols for round rd are ready: project q1/k1 chunk rd
                        # and schedule the a1 sTs whose q/k chunks now exist
                        bg['q'].append(
                            lambda rd=rd: _att_qk_im(nc, qkw1_t, C_QB1, C_KB1, d_t,
                                                     prm_t, q1_t, k1_t, prep, "a1", rd))
                        # only ims 0..1: eT0 has 2 buffers, and im>=2 would
                        # cycle-wait on im0's final-phase feat reads
                        if rd < NCH - 1:
                            for im in range(min(rd + 1, 2)):
                                for jn in range(hi):
                                    if (0, im, jn) not in done_st:
                                        bg['q'].append(st_thunk(0, im, jn))

                    _conv_stream(nc, tc, d['xd'], d['w1d'], d['w2d'],
                                 C_BNS2, C_BNT2, C_C2B2, prm_t, o1p_t, d_t,
                                 wpool, xpool, cps, ctmp, False,
                                 bg=bg, on_round=on_round)
                    while bg['q']:
                        bg['q'].popleft()()

                # conv/prep PSUM freed; feat/z accumulators take their place
                with tc.tile_pool(name="fzp", bufs=2, space="PSUM") as fzp:
                    holder['fz'] = fzp
                    if DEBUG_DUMP:
                        nc.sync.dma_start(d['dbgq'][:], q1_t[:].bitcast(F32))
                        nc.sync.dma_start(d['dbgk'][:], k1_t[:].bitcast(F32))
                        nc.sync.dma_start(d['dbgv'][:], vT1_t[:])
                        nc.sync.dma_start(d['dbgr'][:], r_t[:].bitcast(F32))
                        nc.sync.dma_start(d['dbgd'][:], d_t[:].bitcast(F32))
                    # final phase: remaining sT/exp stream (ims not fully
                    # prefetched) paced 1:1 with feat blocks. a2 blocks and any
                    # a1 pair whose two sTs already ran are ready at slot 0;
                    # the rest join as their rem sTs emit.
                    from collections import deque
                    rem = [(0, im, jn) for im in range(NCH) for jn in range(AJ)
                           if (0, im, jn) not in done_st]
                    rem_idx = {t: i for i, t in enumerate(rem)}
                    ready_now, pending = [], []
                    for im in range(NCH):
                        for j in range(8):
                            need = [(0, im, 2 * j), (0, im, 2 * j + 1)]
                            idxs = [rem_idx[t] for t in need if t in rem_idx]
                            if idxs:
                                pending.append((max(idxs), (0, im, j)))
                            else:
                                ready_now.append((0, im, j))
                    pending.sort()
                    blocks = deque([(1, p // 8, p % 8) for p in range(32)] + ready_now)
                    for i in range(max(64, len(rem))):
                        if i < len(rem):
                            emit_st(*rem[i])
                        while pending and pending[0][0] <= i - 2:
                            blocks.append(pending.pop(0)[1])
                        if blocks:
                            emit_feat(*blocks.popleft())
                    while pending:
                        blocks.append(pending.pop(0)[1])
                    while blocks:
                        emit_feat(*blocks.popleft())

    nc.finalize()
    return nc


def _prep_common(g):
    """Host-side weight layout prep (shared across cores)."""
    out = {}
    for pre, kw1, kw2 in (('sa1', 'w1r', 'w2r'), ('sa2', 'w1d', 'w2d')):
        c1w = g[f'{pre}_c1_w']  # [256, 512, 3, 3]
        c2w = g[f'{pre}_c2_w']  # [512, 256, 3, 3]
        cnp = ml_dtypes.bfloat16 if BF16_CONV else np.float32
        out[kw1] = np.ascontiguousarray(
            c1w.transpose(1, 2, 3, 0).reshape(4, 128, 9, 256).astype(cnp))
        out[kw2] = np.ascontiguousarray(
            c2w.transpose(1, 2, 3, 0).reshape(2, 128, 9, 512).astype(cnp))

    gate = float(g['gate'][0]); beta = float(g['beta'][0]); gamma = float(g['gamma'][0])
    s1 = gate * beta
    s2 = (1.0 - gate) * gamma
    for a, s in ((1, s1), (2, s2)):
        vw = (s * g[f'a{a}_vw']).astype(np.float32)
        qkw = np.concatenate([g[f'a{a}_qw'], g[f'a{a}_kw']], axis=0)  # [64, 256]
        out[f'qkw{a}'] = np.ascontiguousarray(qkw.T.reshape(2, 128, 64))
        out[f'vw{a}'] = np.ascontiguousarray(vw.T.reshape(2, 128, 256))

    prm = np.zeros((128, PRM_COLS), np.float32)
    for pre, cs, ct, cb in (('sa1', C_BNS1, C_BNT1, C_C2B1), ('sa2', C_BNS2, C_BNT2, C_C2B2)):
        s = (g[f'{pre}_bn_g'] / np.sqrt(g[f'{pre}_bn_v'] + EPS)).astype(np.float32)
        t = ((g[f'{pre}_c1_b'] - g[f'{pre}_bn_m']) * s + g[f'{pre}_bn_b']).astype(np.float32)
        prm[:, cs:cs + 2] = s.reshape(2, 128).T
        prm[:, ct:ct + 2] = t.reshape(2, 128).T
        prm[:, cb:cb + 4] = g[f'{pre}_c2_b'].reshape(4, 128).T
    prm[0:32, C_QB1] = g['a1_qb']; prm[0:32, C_KB1] = g['a1_kb']
    prm[0:32, C_QB2] = g['a2_qb']; prm[0:32, C_KB2] = g['a2_kb']
    prm[:, C_VB1:C_VB1 + 2] = (s1 * g['a1_vb']).astype(np.float32).reshape(2, 128).T
    prm[:, C_VB2:C_VB2 + 2] = (s2 * g['a2_vb']).astype(np.float32).reshape(2, 128).T
    prm[0, C_ONESR:C_ONESR + 128] = 1.0
    prm[:, C_ONESC] = 1.0
    out['prm'] = prm
    return out


def _prep_x(x):
    """[512, 44, 44] -> padded [4, 128, 46, 46]."""
    p = np.zeros((512, HP, WP), ml_dtypes.bfloat16 if BF16_CONV else np.float32)
    p[:, 1:45, 1:45] = x
    return p.reshape(4, 128, HP, WP)


_NC_CACHE = None


def kernel(**inputs):
    global _NC_CACHE
    g = {k: np.asarray(v, np.float32) for k, v in inputs.items()}
    if _NC_CACHE is None:
        _NC_CACHE = build_nc()
    nc = _NC_CACHE

    common = _prep_common(g)
    B = g['rgb'].shape[0]
    in_maps = []
    for b in range(B):
        m = dict(common)
        m['xr'] = _prep_x(g['rgb'][b])
        m['xd'] = _prep_x(g['depth'][b])
        in_maps.append(m)

    res = run_bass_kernel_spmd(nc, in_maps, list(range(B)))
    out1 = np.stack([res.results[b]['o1'].reshape(256, H, W) for b in range(B)])
    out2 = np.stack([res.results[b]['o2'].reshape(256, H, W) for b in range(B)])
    return out1, out2

